# revision 29
# baseline (speedup 1.0000x reference)
"""Distributed attention kernel for Trainium2 (8 NeuronCores).

Module: x @ w_qkv -> per-head softmax(q k^T / sqrt(hd)) @ v -> out @ w_proj + b.
Shapes: B=2, N=2048, DIM=1024, H=16, HD=64, f32 in/out; bf16 matmul compute
(f32 PSUM accumulation).

Sharding: core i handles batch b=i//4 and head-group g=i%4 (4 heads).

Per-core structure (v2):
- qT/kT computed feature-major [128, 2048] per head pair; v token-major with
  a fused ones column per head (65 cols) so AV also produces softmax
  denominators.
- S^T per head pair into one PSUM tile [128, 1024] (two 64-row matmuls on
  different PE row groups), exp on ScalarE straight from PSUM (logits O(1):
  no max subtraction).
- AV is q-major: out[q 128, 65] = pt_chunk^T @ [v | ones], accumulated over
  16 k-chunks.  This keeps the matmul free dim at 65 instead of 512, which
  quarters the PE time of the AV stage.  Normalization is a per-partition
  reciprocal + scalar multiply on DVE, then a DMA transpose back to
  feature-major oT for the projection.
- Projection is computed locally as partial sums over the core's own 256
  features: outT_partial[1024 outc, 2048] = wp_ownT @ oT, bias fused on DVE
  (only the g==0 core gets a nonzero bias input).  Partials go to DRAM and
  one ReduceScatter(add) per token half sums them across the 4 cores of the
  batch group, writing each core's [256 outc, 2048] slice directly to the
  output.  This replaces the AllGather + full-contraction projection: the
  only comm left on the critical tail is the second (token half) reduce.
Host assembles the 8 per-core [256, 2048] outT slices into [2, 2048, 1024].
"""

import sys, os

for _p in ("/opt/trn_rl_repo", "/opt/pypackages"):
    if _p not in sys.path:
        sys.path.insert(0, _p)

import numpy as np
import ml_dtypes
from contextlib import ExitStack

import concourse.bass as bass
import concourse.bacc as bacc
import concourse.mybir as mybir
from concourse import tile
from concourse.bass_utils import run_bass_kernel_spmd

F32 = mybir.dt.float32
BF16 = mybir.dt.bfloat16
NPBF16 = np.dtype(ml_dtypes.bfloat16)

P = 128
NTOK = 2048
C = 1024
NH = 4          # heads per core
HD = 64
FEAT = NH * HD  # 256
KT = C // P     # 8 contraction tiles for qkv
MT = NTOK // P  # 16 token tiles
SCALE = HD ** -0.5
N_CORES = 8
GROUPS = [[0, 1, 2, 3], [4, 5, 6, 7]]

AF = mybir.ActivationFunctionType


def build_program(nc):
    xT = nc.dram_tensor("xT", [C, NTOK], BF16, kind="ExternalInput").ap()
    # qkv weights in k-tile-concatenated layout [128, 8*256]:
    # w[p, k*256 + f] = w_orig[k*128 + p, f] -- one DMA per weight
    wq = nc.dram_tensor("wq", [P, KT * FEAT], BF16, kind="ExternalInput").ap()
    wk = nc.dram_tensor("wk", [P, KT * FEAT], BF16, kind="ExternalInput").ap()
    wv = nc.dram_tensor("wv", [P, KT * FEAT], BF16, kind="ExternalInput").ap()
    # own 256 rows of w_proj, all 1024 out cols
    wp = nc.dram_tensor("wp", [FEAT, C], BF16, kind="ExternalInput").ap()
    # bias transposed [128, 8]: bias_t[p, o] = b_proj[o*128 + p]; zeros on
    # cores with g != 0 (bias must be added once per batch group)
    bp = nc.dram_tensor("bp", [P, KT], F32, kind="ExternalInput").ap()
    ident = nc.dram_tensor("ident", [P, P], BF16, kind="ExternalInput").ap()
    bpr = nc.dram_tensor("bpr", [1, C], BF16, kind="ExternalInput").ap()
    # partial projection (outc-major) and the reduce-scattered output slice,
    # split into token halves so each collective's input/output is a fully
    # contiguous DRAM tensor (BIR requires contiguous collective patterns)
    # and the first reduce can fire while the second half is still computing.
    pp = [nc.dram_tensor(f"pp{h}", [C, NTOK // 2], BF16) for h in range(2)]
    # collectives can't target I/O tensors; bounce through internal DRAM
    rso = [nc.dram_tensor(f"rso{h}", [FEAT, NTOK // 2], BF16)
           for h in range(2)]
    # inner dim padded: keeps the final copy's access pattern 2-D (a fully
    # contiguous dst would be lowered to 16KB descriptor chunks and priced
    # ~8x higher); host trims the padding
    out_e = [nc.dram_tensor(f"out{h}", [FEAT, NTOK // 2 + 32], BF16,
                            kind="ExternalOutput").ap() for h in range(2)]

    with tile.TileContext(nc) as tc, ExitStack() as ctx:
        persist = ctx.enter_context(tc.tile_pool(name="persist", bufs=1))
        psum = ctx.enter_context(tc.tile_pool(name="psum", bufs=1, space="PSUM"))
        pa = ctx.enter_context(tc.tile_pool(name="stage_a", bufs=1))
        pb = ctx.enter_context(tc.tile_pool(name="stage_b", bufs=1))

        # ---- PE prewarm: ~3us of dummy matmuls so real work starts at the
        # full clock (the p-state model needs 3us of continuous PE busy).
        warm = persist.tile([P, 512], BF16, tag="warm")
        nc.vector.memset(warm[:], 0.0)
        ps_w = psum.tile([P, 512], F32, tag="ps_mix", bufs=2, name="ps_warm")
        for _ in range(8):
            nc.tensor.matmul(ps_w[:], lhsT=warm[:, 0:P], rhs=warm[:],
                             start=True, stop=True)
        nc.vector.tensor_copy(warm[:], ps_w[:])

        # ---- persistent activations ----
        qT = [persist.tile([P, NTOK], BF16, tag=f"qT{j}", name=f"qT{j}")
              for j in range(2)]
        kT = [persist.tile([P, NTOK], BF16, tag=f"kT{j}", name=f"kT{j}")
              for j in range(2)]
        # v token-major, per head 64 features + a ones column (65 each)
        v_sb = [persist.tile([P, NH * 65], BF16, tag=f"v{m}", name=f"v{m}")
                for m in range(MT)]
        oT = [persist.tile([P, NTOK], BF16, tag=f"oT{j}", name=f"oT{j}")
              for j in range(2)]
        bias_sb = persist.tile([P, KT], F32, tag="bias")
        bpr_sb = persist.tile([1, C], BF16, tag="bpr")
        ones_sb = persist.tile([1, 512], BF16, tag="ones512")
        nc.vector.memset(ones_sb[:], 1.0)
        ident_sb = persist.tile([P, P], BF16, tag="ident")
        wp_sb = [persist.tile([P, C], BF16, tag=f"wp{k}", name=f"wp{k}")
                 for k in range(2)]

        # ---- input loads ----
        # x halves interleaved across SP (k0-3) and Pool (k4-7) so the first
        # token half of every k-tile lands early; weights on ScalarE (idle
        # until the first exp anyway).
        x_sb = [pa.tile([P, NTOK], BF16, tag=f"x{k}", name=f"x{k}")
                for k in range(KT)]
        half = NTOK // 2
        w_sb = {}
        for name in ("wk", "wq", "wv"):
            w_sb[name] = pa.tile([P, KT * FEAT], BF16, tag=name, name=name)
        # wq leads SP, wk leads Act, x split SP(k0-2)/Act(k3)/Pool(k4-7)
        nc.sync.dma_start(w_sb["wq"][:], wq[:])
        nc.scalar.dma_start(w_sb["wk"][:], wk[:])
        for k in range(3):
            nc.sync.dma_start(x_sb[k][:, 0:half], xT[k * P:(k + 1) * P, 0:half])
        nc.scalar.dma_start(x_sb[3][:, 0:half], xT[3 * P:4 * P, 0:half])
        for k in range(4, KT):
            nc.gpsimd.dma_start(x_sb[k][:, 0:half],
                                xT[k * P:(k + 1) * P, 0:half])
        for k in range(3):
            nc.sync.dma_start(x_sb[k][:, half:], xT[k * P:(k + 1) * P, half:])
        nc.scalar.dma_start(x_sb[3][:, half:], xT[3 * P:4 * P, half:])
        for k in range(4, KT):
            nc.gpsimd.dma_start(x_sb[k][:, half:],
                                xT[k * P:(k + 1) * P, half:])
        nc.gpsimd.dma_start(w_sb["wv"][:], wv[:])
        for k in range(2):
            nc.sync.dma_start(wp_sb[k][:], wp[k * P:(k + 1) * P, :])
        nc.scalar.dma_start(bias_sb[:], bp[:])
        nc.scalar.dma_start(bpr_sb[:], bpr[:])
        nc.sync.dma_start(ident_sb[:], ident[:])

        # ---- stage A emitters (gap fill under the exp stream) ----
        def emit_qk(j, names=("wq",), chunks=(0, 1, 2, 3)):
            for wname in names:
                dst = qT if wname == "wq" else kT
                for s in chunks:
                    ps = psum.tile([P, 512], F32, tag="ps_mix", bufs=2,
                                   name="ps_qk")
                    for k in range(KT):
                        nc.tensor.matmul(
                            ps[:],
                            lhsT=w_sb[wname][:, k * FEAT + j * P:
                                             k * FEAT + (j + 1) * P],
                            rhs=x_sb[k][:, s * 512:(s + 1) * 512],
                            start=(k == 0), stop=(k == KT - 1),
                        )
                    nc.vector.tensor_copy(dst[j][:, s * 512:(s + 1) * 512],
                                          ps[:])

        def emit_v(lo=0, hi=MT):
            for m in range(lo, hi):
                ps = psum.tile([P, FEAT], F32, tag="ps_mix", bufs=2,
                               padded_shape=[P, 512], name="ps_v")
                for k in range(KT):
                    nc.tensor.matmul(
                        ps[:],
                        lhsT=x_sb[k][:, m * P:(m + 1) * P],
                        rhs=w_sb["wv"][:, k * FEAT:(k + 1) * FEAT],
                        start=(k == 0), stop=(k == KT - 1),
                    )
                nc.gpsimd.memset(v_sb[m][:], 1.0)
                dst = v_sb[m][:].rearrange("p (h e) -> p h e", e=65)[:, :, 0:64]
                src = ps[:].rearrange("p (h e) -> p h e", e=64)
                nc.vector.tensor_copy(dst, src)

        # ---- stage B: attention ----
        def emit_s_exp(j, s, chain_hooks=None):
            m0 = s * 512
            pt_tiles = []
            for n in range(MT):
                if chain_hooks and n in chain_hooks:
                    chain_hooks[n]()
                ps_s = psum.tile([P, 1024], F32, tag="ps_s", bufs=2)
                for i in range(2):      # head 2j at cols 0:512, 2j+1 after
                    po = i * 64
                    nc.tensor.matmul(
                        ps_s[:, i * 512:(i + 1) * 512],
                        lhsT=kT[j][po:po + 64, n * P:(n + 1) * P],
                        rhs=qT[j][po:po + 64, m0:m0 + 512],
                        start=True, stop=True,
                    )
                pt = pb.tile([P, 1024], BF16, tag="pt", bufs=34)
                nc.scalar.activation(pt[:], ps_s[:], AF.Exp, scale=SCALE)
                pt_tiles.append(pt)
            return pt_tiles

        def emit_av_norm_t(j, s, pt_tiles, tail=False):
            # q-major AV: out[q 128, 65] over 16 k-chunks; col 64 is the
            # softmax denominator.  Normalize on DVE, transpose to oT via
            # DMA transpose on SP.
            m0 = s * 512
            for t in range(4):
                o_dt = F32 if os.environ.get("KM_F32T") else BF16
                o_bf = pb.tile([P, P], o_dt, tag="obf", bufs=8)
                for i in range(2):
                    h = 2 * j + i
                    ps_o = psum.tile([P, 65], F32, tag="ps_o", bufs=2,
                                     padded_shape=[P, 512], name="ps_o")
                    for n in range(MT):
                        nc.tensor.matmul(
                            ps_o[:],
                            lhsT=pt_tiles[n][:, i * 512 + t * P:
                                             i * 512 + (t + 1) * P],
                            rhs=v_sb[n][:, h * 65:(h + 1) * 65],
                            start=(n == 0), stop=(n == MT - 1),
                        )
                    rec = pb.tile([P, 1], F32, tag="rec", bufs=8)
                    nc.vector.reciprocal(rec[:], ps_o[:, 64:65])
                    nc.vector.tensor_scalar_mul(
                        o_bf[:, i * 64:(i + 1) * 64], ps_o[:, 0:64], rec[:])
                # PE transpose back to feature-major (a DMA transpose would
                # be serialized against the collectives by the scheduler)
                ps_t = psum.tile([P, P], o_dt, tag="ps_o", bufs=2,
                                 padded_shape=[P, 1024 if o_dt == BF16 else 512],
                                 name="ps_t")
                nc.tensor.transpose(ps_t[:], o_bf[:], ident_sb[:])
                if tail:    # ScalarE is idle once the exp stream has ended
                    nc.scalar.activation(oT[j][:, m0 + t * P:m0 + (t + 1) * P],
                                         ps_t[:], AF.Copy)
                else:
                    nc.vector.tensor_copy(
                        oT[j][:, m0 + t * P:m0 + (t + 1) * P], ps_t[:])

        # ---- stage C: partial projection (own 256 features, all 1024 outc,
        # outc-major) for one 512-token strip; bias fused on DVE ----
        def emit_proj(s, tail=False):
            for o in range(KT):
                ps_p = psum.tile([P, 512], F32, tag="ps_mix", bufs=2,
                                 name="ps_proj")
                for kk in range(2):
                    nc.tensor.matmul(
                        ps_p[:],
                        lhsT=wp_sb[kk][:, o * P:(o + 1) * P],
                        rhs=oT[kk][:, s * 512:(s + 1) * 512],
                        start=(kk == 0), stop=(kk == 1 and not tail),
                    )
                po_sb = pb.tile([P, 512], BF16, tag="po", bufs=8)
                if tail:
                    # bias via a K=1 matmul so the idle ScalarE can do the
                    # PSUM->SBUF copy (Copy activation takes no AP bias)
                    nc.tensor.matmul(ps_p[:],
                                     lhsT=bpr_sb[0:1, o * P:(o + 1) * P],
                                     rhs=ones_sb[0:1, :],
                                     start=False, stop=True)
                    if o % 2 == 0:
                        nc.scalar.activation(po_sb[:], ps_p[:], AF.Copy)
                    else:
                        nc.vector.tensor_copy(po_sb[:], ps_p[:])
                else:
                    nc.vector.tensor_scalar_add(po_sb[:], ps_p[:],
                                                bias_sb[:, o:o + 1])
                nc.sync.dma_start(pp[s // 2][o * P:(o + 1) * P,
                                             (s % 2) * 512:(s % 2 + 1) * 512],
                                  po_sb[:])

        def emit_rs(h):
            if os.environ.get("KM_NOCC"):
                nc.gpsimd.dma_start(out_e[h][:, :], pp[h][0:FEAT, :])
            else:
                nc.gpsimd.collective_compute(
                    "ReduceScatter",
                    mybir.AluOpType.add,
                    ins=[pp[h][:, :]],
                    outs=[rso[h][:, :]],
                    replica_groups=GROUPS,
                )

        # ---- schedule ----
        emit_qk(0, names=("wk",), chunks=(0,))
        emit_qk(0, names=("wq",), chunks=(0,))
        first_hooks = {
            4: lambda: emit_qk(0, names=("wk",), chunks=(1,)),
            8: lambda: emit_qk(0, names=("wk",), chunks=(2,)),
            12: lambda: emit_qk(0, names=("wk",), chunks=(3,)),
        }
        hooks = {
            0: lambda: (emit_qk(0, names=("wq",), chunks=(1,)), emit_v(0, 8)),
            1: lambda: (emit_qk(0, names=("wq",), chunks=(2,)), emit_v(8, MT)),
            2: lambda: (emit_qk(0, names=("wq",), chunks=(3,)),
                        emit_qk(1, names=("wk",), chunks=(0, 1))),
            3: lambda: (emit_qk(1, names=("wk",), chunks=(2, 3)),
                        emit_qk(1, names=("wq",), chunks=(0,))),
            4: lambda: emit_qk(1, names=("wq",), chunks=(1,)),
            5: lambda: emit_qk(1, names=("wq",), chunks=(2,)),
            6: lambda: emit_qk(1, names=("wq",), chunks=(3,)),
        }
        strips = [(j, s) for j in range(2) for s in range(4)]
        pending = None
        for gi, (j, s) in enumerate(strips):
            pt_tiles = emit_s_exp(j, s, chain_hooks=first_hooks if gi == 0
                                  else None)
            if gi in hooks:
                hooks[gi]()             # PE gap fill under the exp stream
            if pending is not None:
                pj, ps_, ptt = pending
                emit_av_norm_t(pj, ps_, ptt)
                if pj == 1:             # pair-1 strip done -> strip tokens done
                    emit_proj(ps_)
                    if ps_ == 1:
                        emit_rs(0)
                pending = None
            pending = (j, s, pt_tiles)
        # drain the last strip
        pj, ps_, ptt = pending
        emit_av_norm_t(pj, ps_, ptt, tail=True)
        emit_proj(3, tail=True)
        emit_rs(1)
        # copy the reduce-scattered slices out at the very end (own queue
        # position so they can't head-of-line-block anything else)
        tc.no_sync_barrier()
        for h in range(2):
            nc.scalar.dma_start(out_e[h][0:P, 0:NTOK // 2], rso[h][0:P, :])
            nc.sync.dma_start(out_e[h][P:2 * P, 0:NTOK // 2],
                              rso[h][P:2 * P, :])

    return nc


_CACHE = {}


def _get_nc():
    if "nc" not in _CACHE:
        nc = bacc.Bacc("TRN2", target_bir_lowering=False, debug=False,
                       num_devices=N_CORES)
        nc = build_program(nc)
        nc.compile()
        _CACHE["nc"] = nc
    return _CACHE["nc"]


def make_in_maps(x, w_qkv, w_proj, b_proj):
    in_maps = []
    for core in range(N_CORES):
        b, g = core // 4, core % 4
        hs = slice(g * FEAT, (g + 1) * FEAT)
        bias_t = (b_proj.reshape(KT, P).T if g == 0
                  else np.zeros((P, KT), np.float32))
        def cat_w(w):
            # [1024, 256] -> [128, 8*256] with w_cat[p, k*256+f] = w[k*128+p, f]
            return np.ascontiguousarray(
                w.reshape(KT, P, FEAT).transpose(1, 0, 2).reshape(P, KT * FEAT))
        in_maps.append({
            "xT": np.ascontiguousarray(x[b].T).astype(NPBF16),
            "wq": cat_w(w_qkv[:, 0:1024][:, hs]).astype(NPBF16),
            "wk": cat_w(w_qkv[:, 1024:2048][:, hs]).astype(NPBF16),
            "wv": cat_w(w_qkv[:, 2048:3072][:, hs]).astype(NPBF16),
            "wp": np.ascontiguousarray(w_proj[hs, :]).astype(NPBF16),
            "bp": np.ascontiguousarray(bias_t).astype(np.float32),
            "ident": np.eye(P, dtype=np.float32).astype(NPBF16),
            "bpr": (b_proj if g == 0 else np.zeros(C, np.float32)
                    ).reshape(1, C).astype(NPBF16),
        })
    return in_maps


def assemble(results):
    out = np.empty((2, NTOK, 1024), np.float32)
    for core in range(N_CORES):
        b, g = core // 4, core % 4
        for h in range(2):
            out[b][h * 1024:(h + 1) * 1024, g * FEAT:(g + 1) * FEAT] = \
                results[core][f"out{h}"][:, 0:1024].astype(np.float32).T
    return out


def kernel(x, w_qkv, w_proj, b_proj, trace=False):
    nc = _get_nc()
    in_maps = make_in_maps(np.asarray(x), np.asarray(w_qkv),
                           np.asarray(w_proj), np.asarray(b_proj))
    res = run_bass_kernel_spmd(nc, in_maps, core_ids=list(range(N_CORES)),
                               trace=trace)
    out = assemble(res.results)
    if trace:
        return out, res
    return out


# revision 39
# speedup vs baseline: 1.0084x; 1.0084x over previous
"""Distributed attention kernel for Trainium2 (8 NeuronCores).

Module: x @ w_qkv -> per-head softmax(q k^T / sqrt(hd)) @ v -> out @ w_proj + b.
Shapes: B=2, N=2048, DIM=1024, H=16, HD=64, f32 in/out; bf16 matmul compute
(f32 PSUM accumulation).

Sharding: core i handles batch b=i//4 and head-group g=i%4 (4 heads).

Per-core structure (v2):
- qT/kT computed feature-major [128, 2048] per head pair; v token-major with
  a fused ones column per head (65 cols) so AV also produces softmax
  denominators.
- S^T per head pair into one PSUM tile [128, 1024] (two 64-row matmuls on
  different PE row groups), exp on ScalarE straight from PSUM (logits O(1):
  no max subtraction).
- AV is q-major: out[q 128, 65] = pt_chunk^T @ [v | ones], accumulated over
  16 k-chunks.  This keeps the matmul free dim at 65 instead of 512, which
  quarters the PE time of the AV stage.  Normalization is a per-partition
  reciprocal + scalar multiply on DVE, then a DMA transpose back to
  feature-major oT for the projection.
- Projection is computed locally as partial sums over the core's own 256
  features: outT_partial[1024 outc, 2048] = wp_ownT @ oT, bias fused on DVE
  (only the g==0 core gets a nonzero bias input).  Partials go to DRAM and
  one ReduceScatter(add) per token half sums them across the 4 cores of the
  batch group, writing each core's [256 outc, 2048] slice directly to the
  output.  This replaces the AllGather + full-contraction projection: the
  only comm left on the critical tail is the second (token half) reduce.
Host assembles the 8 per-core [256, 2048] outT slices into [2, 2048, 1024].
"""

import sys

for _p in ("/opt/trn_rl_repo", "/opt/pypackages"):
    if _p not in sys.path:
        sys.path.insert(0, _p)

import numpy as np
import ml_dtypes
from contextlib import ExitStack

import concourse.bass as bass
import concourse.bacc as bacc
import concourse.mybir as mybir
from concourse import tile
from concourse.bass_utils import run_bass_kernel_spmd

F32 = mybir.dt.float32
BF16 = mybir.dt.bfloat16
NPBF16 = np.dtype(ml_dtypes.bfloat16)

P = 128
NTOK = 2048
C = 1024
NH = 4          # heads per core
HD = 64
FEAT = NH * HD  # 256
KT = C // P     # 8 contraction tiles for qkv
MT = NTOK // P  # 16 token tiles
SCALE = HD ** -0.5
N_CORES = 8
GROUPS = [[0, 1, 2, 3], [4, 5, 6, 7]]

AF = mybir.ActivationFunctionType


def build_program(nc):
    xT = nc.dram_tensor("xT", [C, NTOK], BF16, kind="ExternalInput").ap()
    # qkv weights in k-tile-concatenated layout [128, 8*256]:
    # w[p, k*256 + f] = w_orig[k*128 + p, f] -- one DMA per weight
    wq = nc.dram_tensor("wq", [P, KT * FEAT], BF16, kind="ExternalInput").ap()
    wk = nc.dram_tensor("wk", [P, KT * FEAT], BF16, kind="ExternalInput").ap()
    wv = nc.dram_tensor("wv", [P, KT * FEAT], BF16, kind="ExternalInput").ap()
    # own 256 rows of w_proj, all 1024 out cols
    wp = nc.dram_tensor("wp", [FEAT, C], BF16, kind="ExternalInput").ap()
    # bias transposed [128, 8]: bias_t[p, o] = b_proj[o*128 + p]; zeros on
    # cores with g != 0 (bias must be added once per batch group)
    bp = nc.dram_tensor("bp", [P, KT], F32, kind="ExternalInput").ap()
    ident = nc.dram_tensor("ident", [P, P], BF16, kind="ExternalInput").ap()
    # partial projection (outc-major) and the reduce-scattered output slice,
    # split into token halves so each collective's input/output is a fully
    # contiguous DRAM tensor (BIR requires contiguous collective patterns)
    # and the first reduce can fire while the second half is still computing.
    pp = [nc.dram_tensor(f"pp{h}", [C, NTOK // 2], BF16) for h in range(2)]
    # collectives can't target I/O tensors; bounce through internal DRAM
    rso = [nc.dram_tensor(f"rso{h}", [FEAT, NTOK // 2], BF16)
           for h in range(2)]
    # inner dim padded: keeps the final copy's access pattern 2-D (a fully
    # contiguous dst would be lowered to 16KB descriptor chunks and priced
    # ~8x higher); host trims the padding
    out_e = [nc.dram_tensor(f"out{h}", [FEAT, NTOK // 2 + 32], BF16,
                            kind="ExternalOutput").ap() for h in range(2)]

    with tile.TileContext(nc) as tc, ExitStack() as ctx:
        persist = ctx.enter_context(tc.tile_pool(name="persist", bufs=1))
        psum = ctx.enter_context(tc.tile_pool(name="psum", bufs=1, space="PSUM"))
        pa = ctx.enter_context(tc.tile_pool(name="stage_a", bufs=1))
        pb = ctx.enter_context(tc.tile_pool(name="stage_b", bufs=1))

        # ---- PE prewarm: ~3us of dummy matmuls so real work starts at the
        # full clock (the p-state model needs 3us of continuous PE busy).
        warm = persist.tile([P, 512], BF16, tag="warm")
        nc.vector.memset(warm[:], 0.0)
        ps_w = psum.tile([P, 512], F32, tag="ps_mix", bufs=2, name="ps_warm")
        for _ in range(8):
            nc.tensor.matmul(ps_w[:], lhsT=warm[:, 0:P], rhs=warm[:],
                             start=True, stop=True)
        nc.vector.tensor_copy(warm[:], ps_w[:])

        # ---- persistent activations ----
        qT = [persist.tile([P, NTOK], BF16, tag=f"qT{j}", name=f"qT{j}")
              for j in range(2)]
        kT = [persist.tile([P, NTOK], BF16, tag=f"kT{j}", name=f"kT{j}")
              for j in range(2)]
        # v token-major, per head 64 features + a ones column (65 each)
        v_sb = [persist.tile([P, NH * 65], BF16, tag=f"v{m}", name=f"v{m}")
                for m in range(MT)]
        oT = [persist.tile([P, NTOK], BF16, tag=f"oT{j}", name=f"oT{j}")
              for j in range(2)]
        bias_sb = persist.tile([P, KT], F32, tag="bias")
        ident_sb = persist.tile([P, P], BF16, tag="ident")
        wp_sb = [persist.tile([P, C], BF16, tag=f"wp{k}", name=f"wp{k}")
                 for k in range(2)]

        # ---- input loads ----
        # x halves interleaved across SP (k0-3) and Pool (k4-7) so the first
        # token half of every k-tile lands early; weights on ScalarE (idle
        # until the first exp anyway).
        x_sb = [pa.tile([P, NTOK], BF16, tag=f"x{k}", name=f"x{k}")
                for k in range(KT)]
        half = NTOK // 2
        w_sb = {}
        for name in ("wk", "wq", "wv"):
            w_sb[name] = pa.tile([P, KT * FEAT], BF16, tag=name, name=name)
        # wq leads SP, wk leads Act, x split SP(k0-2)/Act(k3)/Pool(k4-7)
        nc.sync.dma_start(w_sb["wq"][:], wq[:])
        nc.scalar.dma_start(w_sb["wk"][:], wk[:])
        for k in range(3):
            nc.sync.dma_start(x_sb[k][:, 0:half], xT[k * P:(k + 1) * P, 0:half])
        nc.scalar.dma_start(x_sb[3][:, 0:half], xT[3 * P:4 * P, 0:half])
        for k in range(4, KT):
            nc.gpsimd.dma_start(x_sb[k][:, 0:half],
                                xT[k * P:(k + 1) * P, 0:half])
        for k in range(3):
            nc.sync.dma_start(x_sb[k][:, half:], xT[k * P:(k + 1) * P, half:])
        nc.scalar.dma_start(x_sb[3][:, half:], xT[3 * P:4 * P, half:])
        for k in range(4, KT):
            nc.gpsimd.dma_start(x_sb[k][:, half:],
                                xT[k * P:(k + 1) * P, half:])
        nc.gpsimd.dma_start(w_sb["wv"][:], wv[:])
        for k in range(2):
            nc.sync.dma_start(wp_sb[k][:], wp[k * P:(k + 1) * P, :])
        nc.scalar.dma_start(bias_sb[:], bp[:])
        nc.sync.dma_start(ident_sb[:], ident[:])

        # ---- stage A emitters (gap fill under the exp stream) ----
        def emit_qk(j, names=("wq",), chunks=(0, 1, 2, 3)):
            for wname in names:
                dst = qT if wname == "wq" else kT
                for s in chunks:
                    ps = psum.tile([P, 512], F32, tag="ps_mix", bufs=2,
                                   name="ps_qk")
                    for k in range(KT):
                        nc.tensor.matmul(
                            ps[:],
                            lhsT=w_sb[wname][:, k * FEAT + j * P:
                                             k * FEAT + (j + 1) * P],
                            rhs=x_sb[k][:, s * 512:(s + 1) * 512],
                            start=(k == 0), stop=(k == KT - 1),
                        )
                    nc.vector.tensor_copy(dst[j][:, s * 512:(s + 1) * 512],
                                          ps[:])

        def emit_v(lo=0, hi=MT):
            for m in range(lo, hi):
                ps = psum.tile([P, FEAT], F32, tag="ps_mix", bufs=2,
                               padded_shape=[P, 512], name="ps_v")
                for k in range(KT):
                    nc.tensor.matmul(
                        ps[:],
                        lhsT=x_sb[k][:, m * P:(m + 1) * P],
                        rhs=w_sb["wv"][:, k * FEAT:(k + 1) * FEAT],
                        start=(k == 0), stop=(k == KT - 1),
                    )
                nc.gpsimd.memset(v_sb[m][:], 1.0)
                dst = v_sb[m][:].rearrange("p (h e) -> p h e", e=65)[:, :, 0:64]
                src = ps[:].rearrange("p (h e) -> p h e", e=64)
                nc.vector.tensor_copy(dst, src)

        # ---- stage B: attention ----
        def emit_s_exp(j, s, chain_hooks=None):
            m0 = s * 512
            pt_tiles = []
            for n in range(MT):
                if chain_hooks and n in chain_hooks:
                    chain_hooks[n]()
                ps_s = psum.tile([P, 1024], F32, tag="ps_s", bufs=2)
                for i in range(2):      # head 2j at cols 0:512, 2j+1 after
                    po = i * 64
                    nc.tensor.matmul(
                        ps_s[:, i * 512:(i + 1) * 512],
                        lhsT=kT[j][po:po + 64, n * P:(n + 1) * P],
                        rhs=qT[j][po:po + 64, m0:m0 + 512],
                        start=True, stop=True,
                    )
                pt = pb.tile([P, 1024], BF16, tag="pt", bufs=34)
                nc.scalar.activation(pt[:], ps_s[:], AF.Exp, scale=SCALE)
                pt_tiles.append(pt)
            return pt_tiles

        def emit_av_norm_t(j, s, pt_tiles, tail=False, pe_t=False):
            # q-major AV: out[q 128, 65] over 16 k-chunks; col 64 is the
            # softmax denominator.  Normalize on DVE (reciprocal + scalar
            # multiply), then a PE transpose back to feature-major oT (a DMA
            # transpose would be serialized against the collectives by the
            # scheduler).
            m0 = s * 512
            for t in range(4):
                o_bf = pb.tile([P, P], BF16, tag="obf", bufs=8)
                for i in range(2):
                    h = 2 * j + i
                    ps_o = psum.tile([P, 65], F32, tag="ps_o", bufs=2,
                                     padded_shape=[P, 512], name="ps_o")
                    for n in range(MT):
                        nc.tensor.matmul(
                            ps_o[:],
                            lhsT=pt_tiles[n][:, i * 512 + t * P:
                                             i * 512 + (t + 1) * P],
                            rhs=v_sb[n][:, h * 65:(h + 1) * 65],
                            start=(n == 0), stop=(n == MT - 1),
                        )
                    rec = pb.tile([P, 1], F32, tag="rec", bufs=8)
                    nc.vector.reciprocal(rec[:], ps_o[:, 64:65])
                    nc.vector.tensor_scalar_mul(
                        o_bf[:, i * 64:(i + 1) * 64], ps_o[:, 0:64], rec[:])
                ps_t = psum.tile([P, P], BF16, tag="ps_o", bufs=2,
                                 padded_shape=[P, 1024], name="ps_t")
                nc.tensor.transpose(ps_t[:], o_bf[:], ident_sb[:])
                nc.vector.tensor_copy(
                    oT[j][:, m0 + t * P:m0 + (t + 1) * P], ps_t[:])

        # ---- stage C: partial projection (own 256 features, all 1024 outc,
        # outc-major) for one 512-token strip; bias fused on DVE ----
        def emit_proj(s, tail=False):
            for o in range(KT):
                ps_p = psum.tile([P, 512], F32, tag="ps_mix", bufs=2,
                                 name="ps_proj")
                for kk in range(2):
                    nc.tensor.matmul(
                        ps_p[:],
                        lhsT=wp_sb[kk][:, o * P:(o + 1) * P],
                        rhs=oT[kk][:, s * 512:(s + 1) * 512],
                        start=(kk == 0), stop=(kk == 1),
                    )
                po_sb = pb.tile([P, 512], BF16, tag="po", bufs=8)
                nc.vector.tensor_scalar_add(po_sb[:], ps_p[:],
                                             bias_sb[:, o:o + 1])
                nc.sync.dma_start(pp[s // 2][o * P:(o + 1) * P,
                                             (s % 2) * 512:(s % 2 + 1) * 512],
                                  po_sb[:])

        def emit_rs(h):
            nc.gpsimd.collective_compute(
                "ReduceScatter",
                mybir.AluOpType.add,
                ins=[pp[h][:, :]],
                outs=[rso[h][:, :]],
                replica_groups=GROUPS,
            )

        # ---- schedule ----
        emit_qk(0, names=("wq",), chunks=(0,))
        emit_qk(0, names=("wk",), chunks=(0,))
        first_hooks = {
            4: lambda: emit_qk(0, names=("wk",), chunks=(1,)),
            8: lambda: emit_qk(0, names=("wk",), chunks=(2,)),
            12: lambda: emit_qk(0, names=("wk",), chunks=(3,)),
        }
        hooks = {
            0: lambda: (emit_qk(0, names=("wq",), chunks=(1,)), emit_v(0, 8)),
            1: lambda: (emit_qk(0, names=("wq",), chunks=(2,)), emit_v(8, MT)),
            2: lambda: (emit_qk(0, names=("wq",), chunks=(3,)),
                        emit_qk(1, names=("wk",), chunks=(0, 1))),
            3: lambda: (emit_qk(1, names=("wk",), chunks=(2, 3)),
                        emit_qk(1, names=("wq",), chunks=(0,))),
            4: lambda: emit_qk(1, names=("wq",), chunks=(1,)),
            5: lambda: emit_qk(1, names=("wq",), chunks=(2,)),
            6: lambda: emit_qk(1, names=("wq",), chunks=(3,)),
        }
        strips = [(j, s) for j in range(2) for s in range(4)]
        pending = None
        for gi, (j, s) in enumerate(strips):
            pt_tiles = emit_s_exp(j, s, chain_hooks=first_hooks if gi == 0
                                  else None)
            if gi in hooks:
                hooks[gi]()             # PE gap fill under the exp stream
            if pending is not None:
                pj, ps_, ptt = pending
                emit_av_norm_t(pj, ps_, ptt, pe_t=(pj == 1 and ps_ == 2))
                if pj == 1:             # pair-1 strip done -> strip tokens done
                    emit_proj(ps_)
                    if ps_ == 1:
                        emit_rs(0)
                pending = None
            pending = (j, s, pt_tiles)
        # drain the last strip
        pj, ps_, ptt = pending
        emit_av_norm_t(pj, ps_, ptt, tail=True)
        emit_proj(3, tail=True)
        emit_rs(1)
        # copy the reduce-scattered slices out at the very end (own queue
        # position so they can't head-of-line-block anything else)
        tc.no_sync_barrier()
        for h in range(2):
            nc.scalar.dma_start(out_e[h][0:P, 0:NTOK // 2], rso[h][0:P, :])
            nc.sync.dma_start(out_e[h][P:2 * P, 0:NTOK // 2],
                              rso[h][P:2 * P, :])

    return nc


_CACHE = {}


def _get_nc():
    if "nc" not in _CACHE:
        nc = bacc.Bacc("TRN2", target_bir_lowering=False, debug=False,
                       num_devices=N_CORES)
        nc = build_program(nc)
        nc.compile()
        _CACHE["nc"] = nc
    return _CACHE["nc"]


def make_in_maps(x, w_qkv, w_proj, b_proj):
    in_maps = []
    for core in range(N_CORES):
        b, g = core // 4, core % 4
        hs = slice(g * FEAT, (g + 1) * FEAT)
        bias_t = (b_proj.reshape(KT, P).T if g == 0
                  else np.zeros((P, KT), np.float32))
        def cat_w(w):
            # [1024, 256] -> [128, 8*256] with w_cat[p, k*256+f] = w[k*128+p, f]
            return np.ascontiguousarray(
                w.reshape(KT, P, FEAT).transpose(1, 0, 2).reshape(P, KT * FEAT))
        in_maps.append({
            "xT": np.ascontiguousarray(x[b].T).astype(NPBF16),
            "wq": cat_w(w_qkv[:, 0:1024][:, hs]).astype(NPBF16),
            "wk": cat_w(w_qkv[:, 1024:2048][:, hs]).astype(NPBF16),
            "wv": cat_w(w_qkv[:, 2048:3072][:, hs]).astype(NPBF16),
            "wp": np.ascontiguousarray(w_proj[hs, :]).astype(NPBF16),
            "bp": np.ascontiguousarray(bias_t).astype(np.float32),
            "ident": np.eye(P, dtype=np.float32).astype(NPBF16),
        })
    return in_maps


def assemble(results):
    out = np.empty((2, NTOK, 1024), np.float32)
    for core in range(N_CORES):
        b, g = core // 4, core % 4
        for h in range(2):
            out[b][h * 1024:(h + 1) * 1024, g * FEAT:(g + 1) * FEAT] = \
                results[core][f"out{h}"][:, 0:1024].astype(np.float32).T
    return out


def kernel(x, w_qkv, w_proj, b_proj, trace=False):
    nc = _get_nc()
    in_maps = make_in_maps(np.asarray(x), np.asarray(w_qkv),
                           np.asarray(w_proj), np.asarray(b_proj))
    res = run_bass_kernel_spmd(nc, in_maps, core_ids=list(range(N_CORES)),
                               trace=trace)
    out = assemble(res.results)
    if trace:
        return out, res
    return out
